# revision 25
# baseline (speedup 1.0000x reference)
"""Decode-style single-query attention (B=32, N=8192, D=256, H=8) on 8 TRN2 cores.

Strategy: pure data-parallel over batch (4 batches/core, no collectives).
Per batch, the single query makes K/V projections unnecessary:
  scores[n,h] = X[n,:] @ kq[:,h],  kq = Wk-head-blocks @ (q@Wq + bq)  (bk cancels)
  pooled[h,:] = softmax(scores)[:,h] @ X      (one pass over X)
  attn[e]    = pooled[e//32,:] @ Wv[:,e] + bv[e];  out = q + attn @ Wo + bo

v2: single 8 MB/core fp8 HBM read (x only — no host-pretransposed copy).
The scores matmul needs X^T (d on partitions); it is produced ON-CHIP by
DVE StreamTranspose running on a *uint32 bitcast view* of the fp8 X tile:
4 fp8 lanes ride one u32 element, so the 32x32-block transpose costs 4x
fewer DVE cycles (~512 cyc/slab instead of 2048).  The resulting layout is
quad-interleaved: partition (a,u) holds d=128c+4u+rr at free byte
(j2, 128c+4v+rr) for n=64*(32a+v)+j2.  The scores matmul consumes it with
4 parity (rr) DoubleRow matmuls per slab — moving AP [c(Ko),j,v] with
byte-steps (128,256,4) and offset rr — against block-diagonal stationaries
with kq placed at d=128c+4u+rr (host-precomputed, fp8 with a 2^k gain
undone in the ACT exp scale).  ACT exp emits the softmax denominator via
accum_out and writes fp8 probabilities; a small DVE transpose yields
pooling stationaries; pooling is fp8 DoubleRow over the original X tile.
All 4 batches share one merged bf16/f32 epilogue (denominator select +
normalize + Wv/Wo) with batches packed on partition quadrants (32b + h).
The residual q stays exact via a f32 sidecar folded into qbo = q + bo.

Row map: X row n = 64p + j2 (partition p holds 16 KB contiguous DRAM), so
each batch loads as 2 x 1MB DMAs with 8 KB descriptors.  Engine budget
per core: PE ~28us (scores+pool DR streams), DVE ~27us (xsT-u32 + ptT),
DMA ~24us (8.4 MB @ 358 GB/s), ACT ~8us (exp).  v1 (fp8, host dual-copy
16.8 MB): 78us.
"""

import os
import sys

sys.path.insert(0, "/opt/trn_rl_repo")

from contextlib import ExitStack

import ml_dtypes
import numpy as np

import concourse.bass as bass
import concourse.tile as tile
from concourse import bacc, mybir
from concourse.bass_utils import run_bass_kernel_spmd

F32 = mybir.dt.float32
BF16 = mybir.dt.bfloat16
F8 = mybir.dt.float8e4
U32 = mybir.dt.uint32
NP_F8 = ml_dtypes.float8_e4m3
ts = bass.ts
DR = mybir.MatmulPerfMode.DoubleRow

B, D, H = 32, 256, 8
N = 8192
DH = D // H
NCORES = 8
BL = B // NCORES  # batches per core
SCALE = 1.0 / float(np.sqrt(DH))

NSUB = 64  # j2 columns per batch (n = 64p + j2)
NSLAB = 8  # "slabs" = 8-column groups of the free axis
NGRP = NSLAB // 2  # 2-slab psum/exp groups per batch (4)

EXP = mybir.ActivationFunctionType.Exp

_cache = {}


def build_graph(kqs: float):
    nc = bacc.Bacc("TRN2", target_bir_lowering=False, debug=False, num_devices=NCORES)

    x_ext = nc.declare_dram_parameter("x", [BL, N, D], F8, isOutput=False)
    stat_ext = nc.declare_dram_parameter("stat", [128, BL, 4, 2, 128], F8, isOutput=False)
    qbo_ext = nc.declare_dram_parameter("qbo", [BL, D], F32, isOutput=False)
    sel_ext = nc.declare_dram_parameter("sel", [128, H], F32, isOutput=False)
    mh_ext = nc.declare_dram_parameter("maskh", [128, D], BF16, isOutput=False)
    ones_ext = nc.declare_dram_parameter("ones01", [128, BL], BF16, isOutput=False)
    id16_ext = nc.declare_dram_parameter("ident16", [128, 128], BF16, isOutput=False)
    id32_ext = nc.declare_dram_parameter("ident32", [BL, BL], F32, isOutput=False)
    bvc_ext = nc.declare_dram_parameter("bvc", [128, 2, BL], F32, isOutput=False)
    wv_ext = nc.declare_dram_parameter("wv16", [128, 2, D], BF16, isOutput=False)
    wo_ext = nc.declare_dram_parameter("wo16", [128, 2, D], BF16, isOutput=False)
    out_ext = nc.declare_dram_parameter("out", [BL, D], F32, isOutput=True)

    with tile.TileContext(nc) as tc, ExitStack() as ctx:
        const = ctx.enter_context(tc.tile_pool(name="const", bufs=1))
        xbp = ctx.enter_context(tc.tile_pool(name="xb", bufs=4))
        xsp = ctx.enter_context(tc.tile_pool(name="xs", bufs=4))
        esp = ctx.enter_context(tc.tile_pool(name="es", bufs=3))
        ptp = ctx.enter_context(tc.tile_pool(name="pt", bufs=3))
        lpp = ctx.enter_context(tc.tile_pool(name="lp", bufs=4))
        obp = ctx.enter_context(tc.tile_pool(name="ob", bufs=1))
        spp = ctx.enter_context(tc.tile_pool(name="sp", bufs=3, space="PSUM"))
        accp = ctx.enter_context(tc.tile_pool(name="accp", bufs=2, space="PSUM"))
        eps = ctx.enter_context(tc.tile_pool(name="eps", bufs=2, space="PSUM"))
        wpp = ctx.enter_context(tc.tile_pool(name="wp", bufs=1, space="PSUM"))

        ld = nc.gpsimd  # SWDGE ring for constant/small loads (X owns HWDGE)

        stat_sb = const.tile([128, BL, 4, 2, 128], F8)
        nc.sync.dma_start(stat_sb[:, 0:1], stat_ext.ap()[:, 0:1])  # b0 first
        qbo_sb = const.tile([BL, D], F32)
        sel_sb = const.tile([128, H], F32)
        mh_sb = const.tile([128, D], BF16)
        ones_sb = const.tile([128, BL], BF16)
        id16_sb = const.tile([128, 128], BF16)
        id32_sb = const.tile([BL, BL], F32)
        bvc_sb = const.tile([128, 2, BL], F32)
        wv_sb = const.tile([128, 2, D], BF16)
        wo_sb = const.tile([128, 2, D], BF16)

        def load_consts():
            # stat b1 early (needed ~15us); the rest is gated behind a dummy
            # SBUF->SBUF copy that depends on b1's first x chunk, keeping the
            # SWDGE stream off the wire while the head slabs are in flight.
            ld.dma_start(stat_sb[:, 1:2], stat_ext.ap()[:, 1:2])
            gate = obp.tile([1, 64], F8, tag="gate", name="gate")
            ld.dma_start(gate[:], states[1]["xb"][0:1, 0:1, 0:64])
            ld.dma_start(stat_sb[:, 2:], stat_ext.ap()[:, 2:])
            ld.dma_start(qbo_sb[:], qbo_ext.ap())
            ld.dma_start(sel_sb[:], sel_ext.ap())
            ld.dma_start(mh_sb[:], mh_ext.ap())
            ld.dma_start(ones_sb[:], ones_ext.ap())
            ld.dma_start(id16_sb[:], id16_ext.ap())
            ld.dma_start(id32_sb[:], id32_ext.ap())
            ld.dma_start(bvc_sb[:], bvc_ext.ap())
            ld.dma_start(wv_sb[:], wv_ext.ap())
            ld.dma_start(wo_sb[:], wo_ext.ap())

        states = [dict() for _ in range(BL)]

        def alloc_stream(b, st):
            st["xb"] = xbp.tile([128, NSUB, D], F8, tag="xb", name=f"xb{b}")
            st["xs"] = xsp.tile([128, NSUB, D], F8, tag="xs", name=f"xs{b}")
            st["lparts"] = lpp.tile([128, NGRP], F32, tag="lp", name=f"lp{b}")
            st["sp"] = {}
            st["es"] = {}
            st["pts"] = {}

        def load_batch(b, st, fine=False):
            # row -> partition mapping: n = 64p + j2 (16KB contiguous/partition)
            # Two HWDGE rings stream concurrently (~half wire each); `fine`
            # splits into per-group 256KB chunks so the first group lands in
            # ~1.4us instead of ~5.7us (head-of-pipeline fill).
            src = x_ext.ap()[b].rearrange("(p j) d -> p j d", p=128)
            if fine:
                # slab-sized chunks alternating rings: arrival pitch ~1-1.5us
                # keeps batch-0's PE work dense enough that PE-HAM never sees
                # a >3.4us idle gap (which would re-throttle to 1.2 GHz)
                for lo, hi, ring in (
                    (0, 8, nc.sync),
                    (8, 16, nc.scalar),
                    (16, 24, nc.sync),
                    (24, 32, nc.scalar),
                    (32, 48, nc.sync),
                    (48, 64, nc.scalar),
                ):
                    ring.dma_start(st["xb"][:, lo:hi, :], src[:, lo:hi, :])
            else:
                half = NSUB // 2
                nc.sync.dma_start(st["xb"][:, :half, :], src[:, :half, :])
                nc.scalar.dma_start(st["xb"][:, half:, :], src[:, half:, :])

        def xsT(b, g, st, half=None):
            # 32x32-block transpose on a u32 view: 4 fp8 lanes per element.
            # xs[32a+u, j2, 128c+4v+rr] = xb[32a+v, j2, 128c+4u+rr]
            lo, hi = g * 2 * NSLAB, (g + 1) * 2 * NSLAB
            if half is not None:
                lo, hi = lo + half * NSLAB, lo + (half + 1) * NSLAB
            sl = slice(lo, hi)
            nc.vector.transpose(
                st["xs"][:, sl, :].bitcast(U32),
                st["xb"][:, sl, :].bitcast(U32),
            )

        def _rhs(b, g, sl, st):
            # moving AP for slab sl of group g: dims [c(Ko), j, v], offset rr
            return st["xs"][:, (2 * g + sl) * NSLAB : (2 * g + sl + 1) * NSLAB, :].rearrange(
                "p j (c v r) -> p c j v r", c=2, v=32, r=4
            )

        def scores_slab(b, g, sl, st):
            # one slab's 4-parity accumulation chain: an independent psum
            # group gated only on that slab's arrival + transpose
            if sl == 0:
                st["sp"][g] = spp.tile([128, 2, 256], F32, tag="sp", name=f"sp{b}_{g}")
            sp = st["sp"][g]
            for rr in range(4):
                nc.tensor.matmul(
                    sp[:, sl, :],
                    stat_sb[:, b, rr, :, :],
                    _rhs(b, g, sl, st)[:, :, :, :, rr],
                    start=(rr == 0),
                    stop=(rr == 3),
                    perf_mode=DR,
                    skip_group_check=True,
                )

        def scores_pass(b, rr, gs, st):
            # one stationary load serves all groups in gs (rr-outer emission)
            for g in gs:
                if rr == 0:
                    st["sp"][g] = spp.tile(
                        [128, 2, 256], F32, tag="sp", name=f"sp{b}_{g}"
                    )
                sp = st["sp"][g]
                for sl in range(2):
                    nc.tensor.matmul(
                        sp[:, sl, :],
                        stat_sb[:, b, rr, :, :],
                        _rhs(b, g, sl, st)[:, :, :, :, rr],
                        start=(rr == 0 and sl == 0),
                        stop=(rr == 3 and sl == 1),
                        perf_mode=DR,
                        skip_group_check=True,
                    )

        def expgrp(b, g, st):
            es = esp.tile([128, 2, 256], F8, tag="es", name=f"es{b}_{g}")
            nc.scalar.activation(
                es[:],
                st["sp"].pop(g)[:],
                EXP,
                scale=1.0 / kqs,
                accum_out=st["lparts"][:, g : g + 1],
            )
            st["es"][g] = es

        def ptT(b, g, st):
            pts = ptp.tile([128, 2, 256], F8, tag="pts", name=f"pts{b}_{g}")
            nc.vector.transpose(pts[:], st["es"].pop(g)[:])
            st["pts"][g] = pts

        def pool_grp(b, g, st):
            pts = st["pts"].pop(g)
            lhs = pts.rearrange("p sl (jp t h) -> p sl jp t h", jp=4, t=2)
            for sl in range(2):
                s = 2 * g + sl
                for jp in range(4):
                    base = s * NSLAB + 2 * jp
                    nc.tensor.matmul(
                        st["acc"][:],
                        lhs[:, sl, jp, :, 0:H],
                        st["xb"][:, base : base + 2, :],
                        start=(s == 0 and jp == 0),
                        stop=(s == NSLAB - 1 and jp == 3),
                        perf_mode=DR,
                    )

        pooled16 = obp.tile([128, D], BF16, tag="pooled", name="pooled4")
        nc.vector.memset(pooled16[:], 0.0)

        wsb = obp.tile([128, 128], BF16, tag="warm", name="warm")
        wsb8 = obp.tile([128, 32], F8, tag="warm8", name="warm8")
        wps = wpp.tile([128, 128], F32, tag="warmps", name="warmps")

        def pad(n):
            # dummy matmuls keeping the PE clock warm (PE-HAM re-throttles to
            # 1.2 GHz after ~3.4us idle; warming back costs 2x on real work)
            for _ in range(n):
                nc.tensor.matmul(wps[:], wsb[:], wsb[:], start=True, stop=True)

        def pad_anchored(st, g, sl, n):
            # tiny filler matmuls whose stationary is the just-transposed xs
            # slab slice: they become ready exactly with that slab's real
            # scores, so the scheduler cannot hoist them ahead of the data
            # (which is what sank the plain WAW-chained pad blocks), and they
            # bridge the arrival-paced idle gaps that keep resetting HAM's
            # 3.4us contiguous-busy warm-up window.
            anchor = st["xs"][:, (2 * g + sl) * NSLAB, 0:128]
            for _ in range(n):
                nc.tensor.matmul(
                    wps[:, 0:32], anchor, wsb8[:], start=True, stop=True
                )

        def warmup():
            # ~2.5us of dummy matmuls at body start: PE-HAM flips to 8/8
            # after one busy SHORT window, so real scores run at 2.4 GHz.
            nc.vector.memset(wsb[:], 0.0)
            nc.vector.memset(wsb8[:], 0.0)
            pad(26)

        def normalize(b, st):
            # per-batch: softmax denominator + normalize, frees acc's psum bank
            lsum = obp.tile([128, 1], F32, tag="lsum", name=f"lsum{b}")
            nc.vector.tensor_reduce(
                lsum[:],
                st["lparts"][:],
                axis=mybir.AxisListType.X,
                op=mybir.AluOpType.add,
            )
            lh_ps = eps.tile([H, 1], F32, tag="eps", name=f"lh{b}")
            nc.tensor.matmul(lh_ps[:], sel_sb[:], lsum[:], start=True, stop=True)
            linv = obp.tile([H, 1], F32, tag="linv", name=f"linv{b}")
            nc.vector.reciprocal(linv[:], lh_ps[:])
            nc.vector.tensor_scalar_mul(
                pooled16[32 * b : 32 * b + H, :], st["acc"][:], linv[:, 0:1]
            )

        def epilogue():
            # merged over all 4 batches; batch b packed at partitions 32b+h
            pt_ps = eps.tile([128, 2, 128], BF16, tag="eps", name="ptp4")
            for c in range(2):
                nc.tensor.transpose(
                    pt_ps[:, c, :], pooled16[:, ts(c, 128)], id16_sb[:]
                )
            pt16 = obp.tile([128, 2, 128], BF16, tag="pt16", name="pt16_4")
            nc.vector.tensor_copy(pt16[:], pt_ps[:])

            y_ps = eps.tile([128, D], F32, tag="eps", name="y4")
            for c in range(2):
                nc.tensor.matmul(
                    y_ps[:], pt16[:, c, :], wv_sb[:, c, :], start=(c == 0), stop=(c == 1)
                )
            ym16 = obp.tile([128, D], BF16, tag="ym", name="ym4")
            nc.vector.tensor_mul(ym16[:], y_ps[:], mh_sb[:])

            attn_ps = eps.tile([BL, D], F32, tag="eps", name="attn4")
            nc.tensor.matmul(attn_ps[:], ones_sb[:], ym16[:], start=True, stop=True)
            attn_sb = obp.tile([BL, D], F32, tag="attn", name="attnsb4")
            nc.vector.tensor_copy(attn_sb[:], attn_ps[:])

            at_ps = eps.tile([128, 2, BL], F32, tag="eps", name="at4")
            for c in range(2):
                nc.tensor.transpose(
                    at_ps[:, c, :], attn_sb[:, ts(c, 128)], id32_sb[:]
                )
            at16 = obp.tile([128, 2, BL], BF16, tag="at16", name="at16_4")
            nc.vector.tensor_add(at16[:], at_ps[:], bvc_sb[:])

            res_ps = eps.tile([BL, D], F32, tag="eps", name="res4")
            for c in range(2):
                nc.tensor.matmul(
                    res_ps[:], at16[:, c, :], wo_sb[:, c, :], start=(c == 0), stop=(c == 1)
                )
            out_sb = obp.tile([BL, D], F32, tag="outsb", name="out4")
            nc.vector.tensor_add(out_sb[:], res_ps[:], qbo_sb[:])
            nc.scalar.dma_start(out_ext.ap()[:], out_sb[:])

        # ---- pipelined emission ----
        # Per batch window: PE runs batch b's scores/pool while DVE finishes
        # batch b's ptTs interleaved with batch b+1's xsTs, and HWDGE rings
        # stream batch b+1's x.  DVE FIFO order ptT(b,g), xsT(b+1,g) keeps
        # probs flowing without starving the next batch's transposes.
        for b in range(BL):
            alloc_stream(b, states[b])

        load_batch(0, states[0], fine=True)
        warmup()

        for b in range(BL):
            st = states[b]
            st["acc"] = accp.tile([H, D], F32, tag="acc", name=f"acc{b}")
            nxt = states[b + 1] if b + 1 < BL else None
            if b == 0:
                # no prior window: slab-granular scores chase the arrivals so
                # the PE touches real work every ~1.5us and HAM stays warm
                xsT(b, 0, st, half=0)
                scores_slab(b, 0, 0, st)
                pad_anchored(st, 0, 0, 18)
                if nxt is not None:
                    load_batch(b + 1, nxt)
                xsT(b, 0, st, half=1)
                scores_slab(b, 0, 1, st)
                pad_anchored(st, 0, 1, 18)
                xsT(b, 1, st, half=0)
                scores_slab(b, 1, 0, st)
                pad_anchored(st, 1, 0, 18)
                xsT(b, 1, st, half=1)
                scores_slab(b, 1, 1, st)
                pad_anchored(st, 1, 1, 18)
                load_consts()
                expgrp(b, 0, st)
                ptT(b, 0, st)
                xsT(b, 2, st, half=0)
                scores_slab(b, 2, 0, st)
                pad_anchored(st, 2, 0, 14)
                pool_grp(b, 0, st)
                xsT(b, 2, st, half=1)
                scores_slab(b, 2, 1, st)
                pad_anchored(st, 2, 1, 14)
                expgrp(b, 1, st)
                ptT(b, 1, st)
                xsT(b, 3, st, half=0)
                scores_slab(b, 3, 0, st)
                pad_anchored(st, 3, 0, 10)
                pool_grp(b, 1, st)
                xsT(b, 3, st, half=1)
                scores_slab(b, 3, 1, st)
                pad_anchored(st, 3, 1, 10)
                expgrp(b, 2, st)
                ptT(b, 2, st)
                if nxt is not None:
                    xsT(b + 1, 0, st=nxt)
                    xsT(b + 1, 1, st=nxt)
                pool_grp(b, 2, st)
                expgrp(b, 3, st)
                ptT(b, 3, st)
                if nxt is not None:
                    xsT(b + 1, 2, st=nxt)
                    xsT(b + 1, 3, st=nxt)
            else:
                if nxt is not None:
                    load_batch(b + 1, nxt)
                for rr in range(4):
                    scores_pass(b, rr, range(NGRP), st)
                for g in range(NGRP):
                    expgrp(b, g, st)
                    ptT(b, g, st)
                    if nxt is not None:
                        xsT(b + 1, g, st=nxt)
                    if g >= 1:
                        pool_grp(b, g - 1, st)
            pool_grp(b, NGRP - 1, st)
            normalize(b, st)
        epilogue()

    nc.compile()
    return nc


def _host_prep(inputs):
    x = np.asarray(inputs["x"], dtype=np.float32)
    Wq = np.asarray(inputs["Wq"], dtype=np.float32)
    bq = np.asarray(inputs["bq"], dtype=np.float32)
    Wk = np.asarray(inputs["Wk"], dtype=np.float32)
    Wv = np.asarray(inputs["Wv"], dtype=np.float32)
    Wo = np.asarray(inputs["Wo"], dtype=np.float32)
    bv = np.asarray(inputs["bv"], dtype=np.float32)
    bo = np.asarray(inputs["bo"], dtype=np.float32)
    # bk is unused: softmax is shift-invariant and Q.bk is constant over keys.

    q = np.ascontiguousarray(x[:, 0, :])  # [B, D] f32 (exact residual sidecar)
    qf = q @ Wq + bq  # [B, D]
    # kq[b, d, h] = Wk[d, h-block] . qf[b, h-block], folded softmax scale
    kq = np.einsum(
        "dhm,bhm->bdh", Wk.reshape(D, H, DH), qf.reshape(B, H, DH), optimize=True
    ) * SCALE
    # 2^k gain so fp8 e4m3 holds kq mid-range; undone in the ACT exp scale
    amax = float(np.abs(kq).max())
    kqs = float(2.0 ** np.floor(np.log2(128.0 / max(amax, 1e-30))))
    kq_s = (kq * kqs).astype(NP_F8)

    # block-diagonal stationaries for the quad-interleaved transposed layout:
    # stat[32a+u, b, rr, c, 32a+h] = kq_s[b, 128c+4u+rr, h]
    src = np.asarray(kq_s).reshape(B, 2, 32, 4, H)  # [b, c, u, rr, h]
    stat = np.zeros((128, B, 4, 2, 128), NP_F8)
    per = src.transpose(2, 0, 3, 1, 4)  # [u, b, rr, c, h]
    for a in range(4):
        stat[32 * a : 32 * a + 32, :, :, :, 32 * a : 32 * a + H] = per

    # epilogue constants, batches packed at partitions 32b+h
    e = np.arange(D)
    bh = (np.arange(4)[:, None] * 32 + np.arange(H)[None, :]).ravel()
    sel = np.zeros((128, H), np.float32)
    sel[bh, np.tile(np.arange(H), 4)] = 1.0
    mh128 = np.zeros((128, D), ml_dtypes.bfloat16)
    for b4 in range(BL):
        mh128[32 * b4 : 32 * b4 + H, :] = (
            (np.arange(H)[:, None] == e[None, :] // DH).astype(np.float32)
        ).astype(ml_dtypes.bfloat16)
    ones01 = np.zeros((128, BL), ml_dtypes.bfloat16)
    for b4 in range(BL):
        ones01[32 * b4 : 32 * b4 + H, b4] = 1.0
    bvc4 = np.broadcast_to(
        bv.reshape(2, 128).T[:, :, None], (128, 2, BL)
    ).astype(np.float32)

    shared = {
        "stat": stat,  # sliced per core below
        "qbo": (q + bo).astype(np.float32),  # sliced per core below
        "sel": sel,
        "maskh": mh128,
        "ones01": ones01,
        "ident16": np.eye(128, dtype=ml_dtypes.bfloat16),
        "ident32": np.eye(BL, dtype=np.float32),
        "bvc": np.ascontiguousarray(bvc4),
        "wv16": np.ascontiguousarray(
            Wv.reshape(2, 128, D).transpose(1, 0, 2).astype(ml_dtypes.bfloat16)
        ),
        "wo16": np.ascontiguousarray(
            Wo.reshape(2, 128, D).transpose(1, 0, 2).astype(ml_dtypes.bfloat16)
        ),
    }
    x8 = x.astype(NP_F8)
    return shared, x8, kqs


def kernel(**inputs):
    shared, x8, kqs = _host_prep(inputs)

    key = (kqs,)
    if _cache.get("key") != key:
        _cache["nc"] = build_graph(kqs)
        _cache["key"] = key
    nc = _cache["nc"]

    in_maps = []
    for c in range(NCORES):
        m = {k: v for k, v in shared.items() if k not in ("stat", "qbo")}
        m["stat"] = np.ascontiguousarray(shared["stat"][:, c * BL : (c + 1) * BL])
        m["qbo"] = np.ascontiguousarray(shared["qbo"][c * BL : (c + 1) * BL])
        m["x"] = np.ascontiguousarray(x8[c * BL : (c + 1) * BL])
        in_maps.append(m)

    trace = bool(int(os.environ.get("K_TRACE", "0")))
    res = run_bass_kernel_spmd(
        nc,
        in_maps,
        core_ids=list(range(NCORES)),
        trace=trace,
        tmpdir=os.environ.get("K_TRACE_DIR") or None,
    )
    _cache["last_results"] = res
    out = np.concatenate([res.results[i]["out"] for i in range(NCORES)], axis=0)
    return out.reshape(B, 1, D).astype(np.float32)
